# revision 42
# baseline (speedup 1.0000x reference)
"""BitNet FFN (b1.58) Trainium2 kernel — 8-way Megatron tensor-parallel, v3.

Strategy (hardcoded for x:[4,2048,2048], w_gate/w_up:[8192,2048],
w_down:[2048,8192], subln_weight:[8192], fp32):

  - Shard the intermediate dim I=8192 over 8 cores (I_loc=1024):
    w_gate/w_up row-shards, w_down column-shard, subln_weight shard.
    x is replicated; every core processes all 8192 tokens.
  - All quantization happens on device, matching the reference:
      * activation quant: per-token scale 127/clip(absmax, 1e-5); fp32 RNE
        via the 1.5*2^23 magic-constant trick.
      * weight quant: per-tensor scale 1/clip(mean|w|, 1e-5); global mean
        via three pipelined scalar AllReduces (each overlaps the next
        weight's abs pass).
  - Matmuls: integer-valued bf16 activations x fp8 ternary weights with
    fp32 PSUM accumulation — exact integer arithmetic; dequant scales are
    folded into the PSUM-drain passes.
  - subln stats (per-token sum(T^2), absmax(T*g)) are AllGather'ed once
    per PAIR of 512-token chunks ([2,2,512] f32) to amortize the ~15us
    collective launch overhead.
  - Down-projection partials are combined with ONE bf16 ReduceScatter per
    chunk pair ([1024,2048] bf16 in, [128,2048] bf16 out); final output
    is bf16 (adds ~0.3% error against the 2e-2 budget).
  - Pipeline: iteration ci runs A(ci+1) x-prep, D(ci-3) down+RS, B(ci)
    gate/up+stats (pair AG after odd chunks), C(ci-1) y-quant — B before
    C so the pair AllGather is issued before its even-chunk consumer.
"""
import sys

sys.path.insert(0, "/opt/trn_rl_repo")

import numpy as np

H = 2048
I = 8192
N_CORES = 8
T_TOTAL = 8192
CHUNK = 512
EPS = 1e-5
LN_EPS = 1e-6
C_MAGIC = 12582912.0  # 1.5 * 2**23

_CACHE = {}


def build_nc(h=H, i_full=I, n_cores=N_CORES, t_total=T_TOTAL, chunk=CHUNK,
             repeat=1, no_coll=False, stage_log=None, tune=None):
    from concourse import bacc, tile, mybir
    from concourse import masks

    F32 = mybir.dt.float32
    BF16 = mybir.dt.bfloat16
    FP8 = mybir.dt.float8e4
    AF = mybir.ActivationFunctionType
    ALU = mybir.AluOpType
    AX = mybir.AxisListType

    tn = tune or {}
    i_loc = i_full // n_cores
    kh = h // 128            # contraction tiles for gate/up
    si = i_loc // 128        # contraction tiles for down
    tt_n = chunk // 128      # token tiles per chunk
    nch = t_total // chunk   # chunks
    wi_gu = min(512, i_loc)  # gate/up psum width
    ni = i_loc // wi_gu
    wi_d = min(512, h)       # down psum width
    nh = h // wi_d
    npair = nch // 2
    rs_pair = 2 * chunk // n_cores
    inv_welems = 1.0 / (i_full * h)
    groups = [list(range(n_cores))]

    nc = bacc.Bacc("TRN2", target_bir_lowering=False, debug=False,
                   num_devices=n_cores)

    x_in = nc.dram_tensor("x", [t_total, h], F32, kind="ExternalInput").ap()
    wg_in = nc.dram_tensor("wg", [i_loc, h], F32, kind="ExternalInput").ap()
    wu_in = nc.dram_tensor("wu", [i_loc, h], F32, kind="ExternalInput").ap()
    wd_in = nc.dram_tensor("wd", [h, i_loc], F32, kind="ExternalInput").ap()
    g_in = nc.dram_tensor("g", [1, i_loc], F32, kind="ExternalInput").ap()
    out_ext = nc.dram_tensor("out", [npair * rs_pair, h], BF16,
                             kind="ExternalOutput").ap()

    with tile.TileContext(nc) as tc:
        with (
            tc.tile_pool(name="res", bufs=1) as res,       # persistent
            tc.tile_pool(name="xw", bufs=tn.get("xw", 3)) as xw,
            tc.tile_pool(name="xqw", bufs=tn.get("xqw", 3)) as xqw,
            tc.tile_pool(name="xt", bufs=tn.get("xt", 2)) as xtp,
            tc.tile_pool(name="yq", bufs=tn.get("yq", 12)) as yqp,
            tc.tile_pool(name="ytt", bufs=tn.get("ytt", 3)) as yttp,
            tc.tile_pool(name="zp", bufs=tn.get("zp", 8)) as zp,
            tc.tile_pool(name="scr", bufs=tn.get("scr", 2)) as scr,
            tc.tile_pool(name="osb", bufs=tn.get("osb", 2)) as osbp,
            tc.tile_pool(name="sm", bufs=tn.get("sm", 8)) as sm,
            tc.tile_pool(name="stat", bufs=tn.get("stat", 3)) as statp,
            tc.tile_pool(name="psgu", bufs=tn.get("gu", 4),
                         space="PSUM") as psgu,
            tc.tile_pool(name="psd", bufs=tn.get("pd", 4),
                         space="PSUM") as psd,
            tc.tile_pool(name="dram", bufs=2, space="DRAM") as dramp,
            tc.tile_pool(name="dram1", bufs=1, space="DRAM") as dram1,
        ):
          for _rep in range(repeat):
            # ---------- constants ----------
            ones = res.tile([128, 1], F32)
            nc.gpsimd.memset(ones[:], 1.0)
            lneps = res.tile([128, 1], F32)
            nc.gpsimd.memset(lneps[:], LN_EPS)
            g_rep = res.tile([128, i_loc], F32)
            nc.sync.dma_start(g_rep[:], g_in[:].broadcast_to([128, i_loc]))
            ident = res.tile([128, 128], BF16)
            masks.make_identity(nc, ident[:])

            def mark(lbl):
                if stage_log is not None:
                    blocks = nc.main_func.blocks
                    stage_log.append(
                        (blocks[-1].instructions[-1].name if blocks and
                         blocks[-1].instructions else "I-0", lbl))

            # ---------- pipelined chunk loop stages ----------
            state = {}

            def stage_a(ci):
                """x load + quant + transpose for chunk ci."""
                base = ci * chunk
                invs = sm.tile([128, tt_n], F32, tag="invs", name="invs")
                xq_d = dramp.tile([chunk, h], BF16, tag="xqd", name="xq_d")
                for tt in range(tt_n):
                    xt = xw.tile([128, h], F32, tag="xw", name="xt")
                    nc.sync.dma_start(
                        xt[:],
                        x_in[base + tt * 128: base + (tt + 1) * 128, :])
                    m = sm.tile([128, 1], F32, tag="m", name="m")
                    nc.vector.tensor_reduce(m[:], xt[:], axis=AX.X,
                                            op=ALU.max,
                                            apply_absolute_value=True)
                    nc.vector.tensor_scalar_max(m[:], m[:], EPS)
                    sx = sm.tile([128, 1], F32, tag="sx", name="sx")
                    nc.vector.reciprocal(sx[:], m[:])
                    nc.vector.tensor_scalar_mul(sx[:], sx[:], 127.0)
                    nc.vector.tensor_scalar_mul(invs[:, tt:tt + 1], m[:],
                                                1.0 / 127.0)
                    nc.scalar.activation(xt[:], xt[:], AF.Copy, bias=C_MAGIC,
                                         scale=sx[:])
                    xq = xqw.tile([128, h], BF16, tag="xqw", name="xq")
                    nc.vector.tensor_scalar_add(xq[:], xt[:], -C_MAGIC)
                    nc.sync.dma_start(xq_d[tt * 128:(tt + 1) * 128, :], xq[:])
                kh2 = max(1, kh // 2)
                xqTa = xtp.tile([128, kh2, chunk], BF16, tag="xqTa",
                                name="xqTa")
                xqTb = xtp.tile([128, kh - kh2, chunk], BF16, tag="xqTb",
                                name="xqTb")
                for j in range(kh):
                    dst = xqTa[:, j, :] if j < kh2 else xqTb[:, j - kh2, :]
                    nc.sync.dma_start(dst, xq_d[:, j * 128:(j + 1) * 128],
                                      transpose=True)
                state[ci] = {"invs": invs, "xqT": (xqTa, xqTb, kh2)}
                mark(f"A{ci}")

            def stage_b(ci):
                """gate/up matmuls, A-processing, local stats, pair AG."""
                st_c = state[ci]
                xqTa, xqTb, kh2 = st_c["xqT"]
                cp = ci % 2
                if cp == 0:
                    st_d = dramp.tile([2, 2, chunk], F32, tag="ssd",
                                      name="st_d")
                    state["st_d"] = st_d
                else:
                    st_d = state["st_d"]
                st = statp.tile([128, 2 * tt_n], F32, tag="st", name="st")
                zs = []
                for tt in range(tt_n):
                    pgs, pus = [], []
                    for n in range(ni):
                        pgs.append(psgu.tile([128, wi_gu], F32, tag="gu",
                                             name=f"pg{n}"))
                        pus.append(psgu.tile([128, wi_gu], F32, tag="gu",
                                             name=f"pu{n}"))
                    for k in range(kh):
                        lhs = (xqTa[:, k, tt * 128:(tt + 1) * 128] if k < kh2
                               else xqTb[:, k - kh2, tt * 128:(tt + 1) * 128])
                        for n in range(ni):
                            nc.tensor.matmul(
                                pgs[n][:], lhs,
                                wgqT[k][:, n * wi_gu:(n + 1) * wi_gu],
                                start=(k == 0), stop=(k == kh - 1))
                            nc.tensor.matmul(
                                pus[n][:], lhs,
                                wuqT[k][:, n * wi_gu:(n + 1) * wi_gu],
                                start=(k == 0), stop=(k == kh - 1))
                    z = zp.tile([128, i_loc], F32, tag="z", name="z")
                    for n in range(ni):
                        sl = slice(n * wi_gu, (n + 1) * wi_gu)
                        r = scr.tile([128, wi_gu], F32, tag="scr", name="r")
                        nc.scalar.activation(r[:], pgs[n][:], AF.Relu)
                        nc.vector.tensor_tensor(z[:, sl], r[:], pus[n][:],
                                                op=ALU.mult)
                        nc.vector.tensor_tensor(z[:, sl], z[:, sl], r[:],
                                                op=ALU.mult)
                    # z holds T = U*relu(G)^2 (integer-scaled)
                    sq = scr.tile([128, i_loc], BF16, tag="scr", name="sq")
                    nc.scalar.activation(sq[:], z[:], AF.Square,
                                         accum_out=st[:, tt:tt + 1])
                    nc.vector.tensor_tensor(z[:], z[:], g_rep[:], op=ALU.mult)
                    nc.vector.tensor_reduce(st[:, tt_n + tt:tt_n + tt + 1],
                                            z[:], axis=AX.X, op=ALU.max,
                                            apply_absolute_value=True)
                    zs.append(z)
                nc.sync.dma_start(
                    st_d[:, cp, :].rearrange("s (p t) -> p s t", t=tt_n),
                    st[:].rearrange("p (s t) -> p s t", s=2))
                st_c.update(zs=zs)
                if cp == 1:
                    ag_o = dramp.tile([n_cores, 2, 2, chunk], F32, tag="sso",
                                      name="ag_o")
                    if no_coll:
                        nc.sync.dma_start(ag_o[0], st_d[:])
                    else:
                        nc.gpsimd.collective_compute(
                            "AllGather", ALU.bypass, replica_groups=groups,
                            ins=[st_d[:]], outs=[ag_o[:]])
                    state[ci - 1]["ag_o"] = ag_o
                    st_c["ag_o"] = ag_o
                mark(f"B{ci}")

            def stage_c(ci):
                """stats readback, per-token scalars, y quant."""
                st_c = state[ci]
                invs = st_c["invs"]
                cp = ci % 2
                ag_o = st_c["ag_o"]
                stg = statp.tile([128, n_cores * 2 * tt_n], F32, tag="stg",
                                 name="stg")
                w2 = 2 * tt_n
                for r_ in range(n_cores):
                    src = ag_o[r_ if not no_coll else 0]
                    nc.sync.dma_start(
                        stg[:, r_ * w2:(r_ + 1) * w2].rearrange(
                            "p (s t) -> p s t", s=2),
                        src[:, cp, :].rearrange("s (p t) -> p s t", t=tt_n))
                stv = stg[:].rearrange("p (r s t) -> p s t r", r=n_cores, s=2)
                ssg = statp.tile([128, tt_n], F32, tag="ssg", name="ssg")
                mzg = statp.tile([128, tt_n], F32, tag="mzg", name="mzg")
                nc.vector.tensor_reduce(ssg[:], stv[:, 0], axis=AX.X,
                                        op=ALU.add)
                nc.vector.tensor_reduce(mzg[:], stv[:, 1], axis=AX.X,
                                        op=ALU.max)
                a_t = sm.tile([128, tt_n], F32, tag="a", name="a_t")
                b_t = sm.tile([128, tt_n], F32, tag="b", name="b_t")
                c_t = sm.tile([128, tt_n], F32, tag="c", name="c_t")
                nc.vector.tensor_scalar_mul(a_t[:], invs[:], winv[0])
                nc.vector.tensor_scalar_mul(b_t[:], invs[:], winv[1])
                nc.vector.tensor_tensor(c_t[:], a_t[:], a_t[:], op=ALU.mult)
                nc.vector.tensor_tensor(c_t[:], c_t[:], b_t[:], op=ALU.mult)
                v_t = sm.tile([128, tt_n], F32, tag="v", name="v_t")
                nc.vector.tensor_tensor(v_t[:], ssg[:], c_t[:], op=ALU.mult)
                nc.vector.tensor_tensor(v_t[:], v_t[:], c_t[:], op=ALU.mult)
                c1 = sm.tile([128, tt_n], F32, tag="c1", name="c1")
                nc.scalar.activation(c1[:], v_t[:], AF.Sqrt, bias=lneps[:],
                                     scale=1.0 / i_full)
                nc.vector.reciprocal(c1[:], c1[:])
                ym = sm.tile([128, tt_n], F32, tag="ym", name="ym")
                nc.vector.tensor_tensor(ym[:], mzg[:], c_t[:], op=ALU.mult)
                nc.vector.tensor_tensor(ym[:], ym[:], c1[:], op=ALU.mult)
                nc.vector.tensor_scalar_max(ym[:], ym[:], EPS)
                s_t = sm.tile([128, tt_n], F32, tag="stq", name="s_t")
                nc.vector.reciprocal(s_t[:], ym[:])
                nc.vector.tensor_scalar_mul(s_t[:], s_t[:], 127.0)
                os_t = sm.tile([128, tt_n], F32, tag="os", name="os_t")
                nc.vector.tensor_scalar_mul(os_t[:], ym[:], 1.0 / 127.0)
                nc.vector.tensor_scalar_mul(os_t[:], os_t[:], winv[2])
                cs = sm.tile([128, tt_n], F32, tag="cs", name="cs")
                nc.vector.tensor_tensor(cs[:], c_t[:], c1[:], op=ALU.mult)
                nc.vector.tensor_tensor(cs[:], cs[:], s_t[:], op=ALU.mult)

                yqs = []
                for tt in range(tt_n):
                    z = st_c["zs"][tt]
                    nc.scalar.activation(z[:], z[:], AF.Copy, bias=C_MAGIC,
                                         scale=cs[:, tt:tt + 1])
                    yq = yqp.tile([128, i_loc], BF16, tag="yq", name="yq")
                    nc.vector.tensor_scalar_add(yq[:], z[:], -C_MAGIC)
                    yqs.append(yq)
                st_c.update(yqs=yqs, os_t=os_t)
                mark(f"C{ci}")

            def stage_d(ci):
                """down matmuls, dequant drain, pair bf16 ReduceScatter."""
                st_c = state.pop(ci)
                yqs, os_t = st_c["yqs"], st_c["os_t"]
                cp = ci % 2
                if cp == 0:
                    rs_in = dramp.tile([2 * chunk, h], BF16, tag="rsin",
                                       name="rs_in")
                    state["rs_in"] = rs_in
                else:
                    rs_in = state["rs_in"]
                for tt in range(tt_n):
                    yqT = yttp.tile([128, si, 128], BF16, tag="ytt",
                                    name="yqT")
                    for s in range(si):
                        pt = psd.tile([128, 128], BF16, tag="pd", name="ptd")
                        nc.tensor.transpose(
                            pt[:], yqs[tt][:, s * 128:(s + 1) * 128],
                            ident[:])
                        nc.scalar.copy(yqT[:, s, :], pt[:])
                    ob = osbp.tile([128, h], BF16, tag="osb", name="ob")
                    for n in range(nh):
                        pd = psd.tile([128, wi_d], F32, tag="pd", name="pd")
                        for s in range(si):
                            nc.tensor.matmul(
                                pd[:], yqT[:, s, :],
                                wdqT[s][:, n * wi_d:(n + 1) * wi_d],
                                start=(s == 0), stop=(s == si - 1))
                        nc.scalar.activation(ob[:, n * wi_d:(n + 1) * wi_d],
                                             pd[:], AF.Copy,
                                             scale=os_t[:, tt:tt + 1])
                    nc.gpsimd.dma_start(
                        rs_in[cp * chunk + tt * 128:
                              cp * chunk + (tt + 1) * 128, :], ob[:])
                if cp == 1:
                    p = ci // 2
                    rs_out = dramp.tile([rs_pair, h], BF16, tag="rsout",
                                        name="rs_out")
                    if no_coll:
                        nc.sync.dma_start(rs_out[:], rs_in[0:rs_pair, :])
                    else:
                        nc.gpsimd.collective_compute(
                            "ReduceScatter", ALU.add, replica_groups=groups,
                            ins=[rs_in[:]], outs=[rs_out[:]])
                    nc.gpsimd.dma_start(
                        out_ext[p * rs_pair:(p + 1) * rs_pair, :], rs_out[:])
                mark(f"D{ci}")

            # ---------- weight pipeline (per-weight abs -> AR pipelined) ---
            w_list = [(wg_in, i_loc), (wu_in, i_loc), (wd_in, h)]
            swq = [None, None, None]   # [128,1] quant scale per tensor
            winv = [None, None, None]  # [128,1] dequant scale per tensor

            def weight_abs():
                for idx, (w_ap, rows) in enumerate(w_list):
                    cols = w_ap.shape[1]
                    acc = sm.tile([128, 1], F32, tag="acc", name=f"acc{idx}")
                    nc.gpsimd.memset(acc[:], 0.0)
                    for t in range(rows // 128):
                        tag, pool = (("xw", xw), ("xqTa", xtp),
                                     ("xqTb", xtp))[t % 3]
                        wt = pool.tile([128, cols], F32, tag=tag,
                                       name=f"wabs{idx}")
                        nc.sync.dma_start(wt[:],
                                          w_ap[t * 128:(t + 1) * 128, :])
                        sct = scr.tile([128, wi_gu], BF16, tag="scr",
                                       name=f"sct{idx}")
                        for c0 in range(0, cols, wi_gu):
                            pacc = sm.tile([128, 1], F32, tag="pacc",
                                           name=f"pacc{idx}")
                            nc.scalar.activation(sct[:], wt[:, c0:c0 + wi_gu],
                                                 AF.Abs, accum_out=pacc[:])
                            nc.vector.tensor_tensor(acc[:], acc[:], pacc[:],
                                                    op=ALU.add)
                    ps1 = psd.tile([1, 1], F32, tag="pd", name=f"ps1_{idx}")
                    nc.tensor.matmul(ps1[:], acc[:], ones[:], start=True,
                                     stop=True)
                    s1 = sm.tile([1, 1], F32, tag="s1", name=f"s1_{idx}")
                    nc.scalar.copy(s1[:], ps1[:])
                    ws_d = dram1.tile([1, 1], F32, tag=f"wsd{idx}",
                                      name=f"wsd{idx}")
                    nc.sync.dma_start(ws_d[:], s1[:])
                    ws_o = dram1.tile([1, 1], F32, tag=f"wso{idx}",
                                      name=f"wso{idx}")
                    if no_coll:
                        nc.sync.dma_start(ws_o[:], ws_d[:])
                    else:
                        nc.gpsimd.collective_compute(
                            "AllReduce", ALU.add, replica_groups=groups,
                            ins=[ws_d[:]], outs=[ws_o[:]])
                    # readback + scale plumbing on the Pool queue so the SP
                    # queue keeps streaming the next weight's loads while the
                    # AllReduce is in flight
                    wsl = sm.tile([1, 2], F32, tag="wsl", name=f"wsl{idx}")
                    nc.gpsimd.dma_start(wsl[:, 0:1], ws_o[:])
                    nc.vector.tensor_scalar(out=wsl[:, 0:1], in0=wsl[:, 0:1],
                                            scalar1=inv_welems, scalar2=EPS,
                                            op0=ALU.mult, op1=ALU.max)
                    nc.vector.reciprocal(wsl[:, 1:2], wsl[:, 0:1])
                    sc_d = dram1.tile([1, 2], F32, tag=f"scd{idx}",
                                      name=f"scd{idx}")
                    nc.gpsimd.dma_start(sc_d[:], wsl[:])
                    swt = res.tile([128, 2], F32, name=f"swt{idx}")
                    nc.gpsimd.dma_start(swt[:],
                                        sc_d[:].broadcast_to([128, 2]))
                    winv[idx] = swt[:, 0:1]
                    swq[idx] = swt[:, 1:2]
                    mark(f"wabs{idx}")

            weight_abs()

            wT = [[], [], []]

            def weight_quant():
                for idx, (w_ap, rows) in enumerate(w_list):
                    cols = w_ap.shape[1]
                    nslab, slabw = (kh, i_loc) if idx < 2 else (si, h)
                    for j in range(nslab):
                        sl8 = res.tile([128, slabw], FP8, name=f"wT{idx}_{j}")
                        wT[idx].append(sl8)
                    for t in range(rows // 128):
                        if idx == 0:
                            tag, pool = (("xw", xw), ("xqTa", xtp),
                                         ("xqTb", xtp))[t % 3]
                        else:
                            tag, pool = "xw", xw
                        wt = pool.tile([128, cols], F32, tag=tag,
                                       name=f"wqt{idx}")
                        nc.sync.dma_start(wt[:],
                                          w_ap[t * 128:(t + 1) * 128, :])
                        nc.scalar.activation(wt[:], wt[:], AF.Copy,
                                             bias=C_MAGIC, scale=swq[idx])
                        nc.vector.tensor_scalar(
                            out=wt[:], in0=wt[:], scalar1=C_MAGIC + 1.0,
                            scalar2=C_MAGIC - 1.0, op0=ALU.min, op1=ALU.max)
                        wqt = xqw.tile([128, cols], BF16, tag="xqw",
                                       name=f"wqq{idx}")
                        nc.vector.tensor_scalar_add(wqt[:], wt[:], -C_MAGIC)
                        for j in range(nslab):
                            pt = psd.tile([128, 128], BF16, tag="pd",
                                          name=f"pt{idx}")
                            nc.tensor.transpose(pt[:],
                                                wqt[:, j * 128:(j + 1) * 128],
                                                ident[:])
                            nc.vector.tensor_copy(
                                wT[idx][j][:, t * 128:(t + 1) * 128], pt[:])
                    mark(f"wquant{idx}")
                    if idx == 0:
                        stage_a(0)
                    if idx == 1:
                        stage_a(1)

            weight_quant()
            wgqT, wuqT, wdqT = wT

            # ---------- main pipelined loop ----------
            def main_loop():
                for it in range(nch + 4):
                    # D first: its ACT PSUM-drain copies must not queue
                    # behind A's activation-quant rounds (in-order ACT)
                    if it >= 3 and it - 3 < nch:
                        stage_d(it - 3)
                    if 2 <= it + 1 < nch:
                        stage_a(it + 1)
                    if it < nch:
                        stage_b(it)
                    if it >= 1 and it - 1 < nch:
                        stage_c(it - 1)

            main_loop()

    nc.compile()
    return nc


def _get_nc(key, **kw):
    if key not in _CACHE:
        _CACHE[key] = build_nc(**kw)
    return _CACHE[key]


def kernel(x, w_gate, w_up, w_down, subln_weight):
    from concourse.bass_utils import run_bass_kernel_spmd

    nc = _get_nc("full")
    x2 = np.ascontiguousarray(np.asarray(x, np.float32).reshape(T_TOTAL, H))
    i_loc = I // N_CORES
    in_maps = []
    for c in range(N_CORES):
        sl = slice(c * i_loc, (c + 1) * i_loc)
        in_maps.append({
            "x": x2,
            "wg": np.ascontiguousarray(np.asarray(w_gate, np.float32)[sl, :]),
            "wu": np.ascontiguousarray(np.asarray(w_up, np.float32)[sl, :]),
            "wd": np.ascontiguousarray(np.asarray(w_down, np.float32)[:, sl]),
            "g": np.ascontiguousarray(
                np.asarray(subln_weight, np.float32).reshape(1, I)[:, sl]),
        })
    res = run_bass_kernel_spmd(nc, in_maps, list(range(N_CORES)))
    npair = T_TOTAL // (2 * CHUNK)
    rs_pair = 2 * CHUNK // N_CORES
    full = np.empty((npair, N_CORES, rs_pair, H), np.float32)
    for c in range(N_CORES):
        full[:, c] = res.results[c]["out"].astype(np.float32).reshape(
            npair, rs_pair, H)
    return full.reshape(4, 2048, H)
